# revision 13
# baseline (speedup 1.0000x reference)
"""Trainium2 Bass kernel for nn_Beta_score2 (gnn_message_passing).

Computation (per batch element b):
  nodes   = 6 feature vectors x_k (padded to 2048; padding never contributes)
  temp_k  = tanh(x_k @ W[:, :d_k]^T + b)          # [512]
  score_k = temp_k . h_n                           # scalar
  beta    = softmax(score)                         # [6]
  z       = sum_k beta_k * pad(x_k)                # [2048], cols 1024: always 0

Sharding: data-parallel over batch, B=8192 -> 1024 per core on 8 cores.

Per-core pipeline (two 512-wide batch chunks):
  stage 1: PE matmuls W^T-chunks x xT-chunks -> PSUM [128o, 512b];
           ACT fused bias+tanh -> temp^T in SBUF (bf16; PE runs bf16 at the
           full 2.4 GHz while fp16 is throttled ~20%).
  stage 2: score matmuls use 128-col zero-padded h-tiles (enables FWL fast
           weight load -> big-matmul speed) accumulating all 24 (node, oc)
           pieces into one PSUM [128, 512] whose rows 0:6 are the scores;
           PE-transpose to batch-major, softmax on ACT/DVE -> beta [128b, 24].
  stage 3: batch-major weighted sum with beta as per-partition scalars:
           ACT z = copy(x0 * b0), DVE scalar_tensor_tensor z += bk * xk.
           x for this stage (xb, fp16) is the natural row-major layout so its
           DMA is fully contiguous; z leaves batch-major (no host transpose).
  head: PE warm-up junk matmuls run during the initial DMA wait so the HAM
        clock gate releases before real matmuls; chunk-0 stage-2/3 emission
        is deferred past chunk-1's first node so the PE queue never blocks
        on the softmax transposes.

Host pre-tiles every DRAM tensor so each dma_start is a contiguous copy.
"""

import os
import sys
import types

import numpy as np

B_TOTAL = 8192
NCORES = 8
BLOC = B_TOTAL // NCORES  # 1024
OUT = 512
DW = 1024                 # only W[:, :1024] is ever used
NODES = 6
NODE_OFF = [0, 1024, 1536, 2048, 2560, 3584]
NODE_DIM = [1024, 512, 512, 512, 1024, 512]
NODE_ORDER = (1, 0, 4, 2, 3, 5)   # 1 first (single x-group), then 8-kc nodes
                                  # so the tanh pipeline gets slack
BC = 512                  # batch chunk on the free dim
NBC = BLOC // BC
GK = 4                    # xT group: [128, GK, BC]
NG = 8                    # 32 kc-chunks / GK

MM_DTYPE = os.environ.get("KERNEL_MM_DTYPE", "bfloat16")
S3_DTYPE = os.environ.get("KERNEL_S3_DTYPE", "float16")
N_JUNK = int(os.environ.get("KERNEL_N_JUNK", "11"))

LAST_EXEC_TIME_NS = None
LAST_RESULT = None

_cache = {}


def _install_ntff_hook():
    """run_bass_kernel_spmd(trace=True) under axon needs antenv.axon_hooks,
    which this image lacks; synthesize it from trn_agent_boot."""
    if "antenv.axon_hooks" in sys.modules:
        return
    try:
        import antenv
        import trn_agent_boot.trn_boot as tb
    except Exception:
        return
    mod = types.ModuleType("antenv.axon_hooks")
    _hook = tb._ntff_profile_via_ctypes("/opt/axon/libaxon_pjrt.so")
    mod.get_axon_ntff_profile_hook = lambda: _hook
    mod.set_axon_ntff_profile_hook = lambda h: None
    sys.modules["antenv.axon_hooks"] = mod
    antenv.axon_hooks = mod


def _build(mm_dtype_name, s3_dtype_name):
    from contextlib import ExitStack

    import concourse.bacc as bacc
    import concourse.mybir as mybir
    import concourse.tile as tile

    f32 = mybir.dt.float32
    mm_dt = getattr(mybir.dt, mm_dtype_name)
    s3_dt = getattr(mybir.dt, s3_dtype_name)

    nc = bacc.Bacc("TRN2", target_bir_lowering=False, debug=False)
    # pre-tiled inputs: every dma_start below is a contiguous copy
    xt_d = nc.dram_tensor("xt", [NBC, NG, 128, GK, BC], mm_dt, kind="ExternalInput").ap()
    xb_d = nc.dram_tensor("xb", [8, 128, 4096], s3_dt, kind="ExternalInput").ap()
    wt_d = nc.dram_tensor("wt", [2, 128, 4 * OUT], mm_dt, kind="ExternalInput").ap()
    bias_d = nc.dram_tensor("bias", [128, 4], f32, kind="ExternalInput").ap()
    h6_d = nc.dram_tensor("h6", [128, 24, 128], mm_dt, kind="ExternalInput").ap()
    eye_d = nc.dram_tensor("eye", [6, 6], f32, kind="ExternalInput").ap()
    eye128_d = nc.dram_tensor("eye128", [128, 128], s3_dt, kind="ExternalInput").ap()
    z_d = nc.dram_tensor("z", [8, 128, DW], s3_dt, kind="ExternalOutput").ap()

    Tanh = mybir.ActivationFunctionType.Tanh
    Exp = mybir.ActivationFunctionType.Exp
    Copy = mybir.ActivationFunctionType.Copy
    Mult = mybir.AluOpType.mult
    Add = mybir.AluOpType.add

    with tile.TileContext(nc) as tc, ExitStack() as ctx:
        const = ctx.enter_context(tc.tile_pool(name="const", bufs=1))
        wt_lo = const.tile([128, 4, OUT], mm_dt)
        wt_hi = const.tile([128, 4, OUT], mm_dt)
        bias_t = const.tile([128, 4], f32)
        h6_t = const.tile([128, 24, 128], mm_dt)
        eye_t = const.tile([6, 6], f32)
        eye128_t = const.tile([128, 128], s3_dt)
        junk_w = const.tile([128, 128], mm_dt, name="junk_w")
        junk_x = const.tile([128, BC], mm_dt, name="junk_x")

        pre_ps = ctx.enter_context(tc.tile_pool(name="pre", bufs=4, space="PSUM"))
        score_ps = ctx.enter_context(tc.tile_pool(name="score", bufs=2, space="PSUM"))
        tp_ps = ctx.enter_context(tc.tile_pool(name="tp", bufs=1, space="PSUM"))
        temps = ctx.enter_context(tc.tile_pool(name="temps", bufs=4))
        small = ctx.enter_context(tc.tile_pool(name="small", bufs=2))
        zpool = ctx.enter_context(tc.tile_pool(name="zpool", bufs=3))
        diag_pool = ctx.enter_context(tc.tile_pool(name="diag", bufs=12))

        # ---- PE warm-up: junk matmuls release the HAM clock gate while the
        # first DMAs are in flight.  memsets on GpSimd (idle at start).
        nc.gpsimd.memset(junk_w[:], 0.0)
        nc.gpsimd.memset(junk_x[:], 0.0)
        jp = score_ps.tile([128, BC], f32, name="junk_ps", tag="sc")
        for i in range(N_JUNK):
            nc.tensor.matmul(jp[:], junk_w[:], junk_x[:], start=True, stop=True)
        # dummy tanh pulls the 1.3us ACT table load into the DMA-wait window
        warm_t = temps.tile([128, BC], mm_dt, tag="tt", name="warm_t")
        nc.scalar.activation(warm_t[:], junk_x[:], Tanh, bias=0.0, scale=1.0)

        # ---- DMA dispatch. Critical path on Sync; bulk x streams on GpSimd.
        xts_tiles = {}

        def load_xt(bc, g, eng):
            t = const.tile([128, GK, BC], mm_dt, name=f"xt_{bc}_{g}")
            eng.dma_start(t[:], xt_d[bc, g])
            xts_tiles[(g, bc)] = t

        xb_tiles = {}

        def load_xb(j, eng):
            t = const.tile([128, 4096], s3_dt, name=f"xb_{j}")
            eng.dma_start(t[:], xb_d[j])
            xb_tiles[j] = t

        # critical path on the Sync queue only (MM#1's semaphore wait is
        # batched per-queue, so nothing else may ride on it); h6/eyes go on
        # the Scalar queue, bulk x on GpSimd.
        nc.sync.dma_start(wt_lo[:], wt_d[0])
        load_xt(0, 2, nc.sync)
        nc.sync.dma_start(wt_hi[:], wt_d[1])
        nc.sync.dma_start(bias_t[:], bias_d[:, :])
        nc.scalar.dma_start(h6_t[:], h6_d[:, :, :])
        nc.scalar.dma_start(eye_t[:], eye_d[:, :])
        nc.scalar.dma_start(eye128_t[:], eye128_d[:, :])
        for g in (0, 1, 5, 6, 3, 4, 7):
            load_xt(0, g, nc.gpsimd)
        for g in (2, 0, 1):
            load_xt(1, g, nc.gpsimd)
        for j in range(4):
            load_xb(j, nc.gpsimd)
        for g in (5, 6, 3, 4, 7):
            load_xt(1, g, nc.gpsimd)
        for j in range(4, 8):
            load_xb(j, nc.gpsimd)

        def xts(kc, bc):
            return xts_tiles[(kc // GK, bc)][:, kc % GK, :]

        def wts(kc, oc):
            w = wt_lo if kc < 4 else wt_hi
            return w[:, kc % 4, oc * 128 : (oc + 1) * 128]

        # Score matmuls are emitted one group late ("pending"), so the PE
        # always has the next group's main matmuls queued between a tanh and
        # the score matmul that consumes it.
        pending_sc = []

        def flush_sc():
            if pending_sc:
                sc_t, lhsT, rhs, st, sp = pending_sc.pop()
                nc.tensor.matmul(sc_t[:], lhsT, rhs, start=st, stop=sp)

        scs = {}

        def emit_stage1_node(bc, n):
            if n == NODE_ORDER[0]:
                scs[bc] = score_ps.tile([128, BC], f32, name=f"sc_{bc}", tag="sc")
            sc = scs[bc]
            nk = NODE_DIM[n] // 128
            off = NODE_OFF[n] // 128
            for oc in range(4):
                ps = pre_ps.tile([128, BC], f32)
                for kc in range(nk):
                    nc.tensor.matmul(
                        ps[:],
                        wts(kc, oc),
                        xts(off + kc, bc),
                        start=(kc == 0),
                        stop=(kc == nk - 1),
                    )
                tt = temps.tile([128, BC], mm_dt, tag="tt", name="tt")
                nc.scalar.activation(
                    tt[:], ps[:], Tanh, bias=bias_t[:, oc : oc + 1], scale=1.0
                )
                flush_sc()
                pending_sc.append(
                    (
                        sc,
                        h6_t[:, n * 4 + oc, :],
                        tt[:],
                        (n == NODE_ORDER[0] and oc == 0),
                        (n == NODE_ORDER[-1] and oc == 3),
                    )
                )

        def emit_stage23(bc):
            sc = scs[bc]
            # ---------- softmax over the 6 nodes (batch-major) ----------
            sc_sb = small.tile([6, BC], f32, tag="sc_sb")
            nc.scalar.copy(sc_sb[:], sc[0:6, :])
            tp = tp_ps.tile([128, 24], f32)
            for j in range(4):
                nc.tensor.transpose(
                    tp[:, j * 6 : (j + 1) * 6],
                    sc_sb[:, j * 128 : (j + 1) * 128],
                    eye_t[:],
                )
            expt = small.tile([128, 24], f32, tag="expt")
            sumexp = small.tile([128, 4], f32, tag="sumexp")
            nc.scalar.activation(expt[:], tp[:], Exp)
            nc.vector.tensor_reduce(
                sumexp[:],
                expt[:].rearrange("p (j k) -> p j k", j=4),
                axis=mybir.AxisListType.X,
                op=mybir.AluOpType.add,
            )
            rec = small.tile([128, 4], f32, tag="rec")
            nc.vector.reciprocal(rec[:], sumexp[:])
            beta = small.tile([128, 24], f32, tag="beta")
            for j in range(4):
                nc.vector.tensor_scalar_mul(
                    beta[:, j * 6 : (j + 1) * 6],
                    expt[:, j * 6 : (j + 1) * 6],
                    rec[:, j : j + 1],
                )
            # ---------- stage 3: batch-major z = sum_k beta_k * x_k ----------
            def dve_group(j):
                # ACT z-init + DVE FMA chain
                xb = xb_tiles[bc * 4 + j]
                bf = beta[:, j * 6 : j * 6 + 6]
                z = zpool.tile([128, DW], s3_dt, tag="z", name=f"z_{bc}_{j}")
                nc.scalar.activation(z[:], xb[:, 0:DW], Copy, scale=bf[:, 0:1])
                nc.vector.scalar_tensor_tensor(
                    z[:], xb[:, 2560:3584], bf[:, 4:5], z[:], Mult, Add
                )
                for k, lo in ((1, 1024), (2, 1536), (3, 2048), (5, 3584)):
                    nc.vector.scalar_tensor_tensor(
                        z[:, 0:512], xb[:, lo : lo + 512], bf[:, k : k + 1],
                        z[:, 0:512], Mult, Add,
                    )
                nc.sync.dma_start(z_d[bc * 4 + j], z[:])

            if bc < NBC - 1:
                for j in range(4):
                    dve_group(j)
            else:
                # tail chunk: j3 on ACT+DVE; j0/j1/j2 as PE diag-matmuls
                # (diag tiles for j0 built on DVE, j1/j2 on ACT); copies
                # back from PSUM split ACT (j0, j2) / DVE (j1).
                dve_group(3)

                def build_diags(j, eng):
                    bf = beta[:, j * 6 : j * 6 + 6]
                    diags = []
                    for k in range(6):
                        d = diag_pool.tile([128, 128], s3_dt, tag="dg", name=f"dg{j}_{k}")
                        if eng == "dve":
                            nc.vector.tensor_scalar_mul(
                                d[:], eye128_t[:], bf[:, k : k + 1]
                            )
                        else:
                            nc.scalar.activation(
                                d[:], eye128_t[:], Copy, scale=bf[:, k : k + 1]
                            )
                        diags.append(d)
                    return diags

                def pe_group_mm(j, diags):
                    xb = xb_tiles[bc * 4 + j]
                    za = pre_ps.tile([128, BC], f32, tag="ps", name=f"za_{j}")
                    for i, (k, lo) in enumerate(
                        ((0, 0), (1, 1024), (2, 1536), (3, 2048), (4, 2560), (5, 3584))
                    ):
                        nc.tensor.matmul(
                            za[:], diags[k][:], xb[:, lo : lo + 512],
                            start=(i == 0), stop=(i == 5),
                        )
                    zb = pre_ps.tile([128, BC], f32, tag="ps", name=f"zb_{j}")
                    nc.tensor.matmul(
                        zb[:], diags[0][:], xb[:, 512:1024], start=True, stop=False
                    )
                    nc.tensor.matmul(
                        zb[:], diags[4][:], xb[:, 3072:3584], start=False, stop=True
                    )
                    return za, zb

                def pe_group_out(j, za, zb, eng):
                    z = zpool.tile([128, DW], s3_dt, tag="z", name=f"zd_{j}")
                    if eng == "dve":
                        nc.vector.tensor_copy(z[:, 0:512], za[:])
                        nc.vector.tensor_copy(z[:, 512:1024], zb[:])
                    else:
                        nc.scalar.copy(z[:, 0:512], za[:])
                        nc.scalar.copy(z[:, 512:1024], zb[:])
                    nc.sync.dma_start(z_d[bc * 4 + j], z[:])

                d0 = build_diags(0, "dve")
                za0, zb0 = pe_group_mm(0, d0)
                d1 = build_diags(1, "act")
                za1, zb1 = pe_group_mm(1, d1)
                d2 = build_diags(2, "act")
                za2, zb2 = pe_group_mm(2, d2)
                pe_group_out(0, za0, zb0, "act")
                pe_group_out(1, za1, zb1, "dve")
                pe_group_out(2, za2, zb2, "act")

        # chunk 0 stage 1; defer its stage 2/3 past chunk 1's first node so
        # the softmax transposes never block the PE queue.
        for n in NODE_ORDER:
            emit_stage1_node(0, n)
        emit_stage1_node(1, NODE_ORDER[0])
        emit_stage23(0)
        for n in NODE_ORDER[1:]:
            emit_stage1_node(1, n)
        flush_sc()
        # keep the PE busy through the softmax window so the HAM clock gate
        # stays open for the tail diag-matmuls
        jp2 = pre_ps.tile([128, BC], f32, tag="ps", name="jp2")
        for i in range(10):
            nc.tensor.matmul(jp2[:], junk_w[:], junk_x[:], start=True, stop=True)
        emit_stage23(1)

    nc.compile()
    return nc


def _get_nc():
    key = (MM_DTYPE, S3_DTYPE)
    if key not in _cache:
        _cache[key] = _build(*key)
    return _cache[key]


def kernel(result_ls, result_A, result_lm, result_AT, result_ds, result_dm, W, b, h_n):
    global LAST_EXEC_TIME_NS, LAST_RESULT
    _install_ntff_hook()
    from concourse.bass_utils import run_bass_kernel_spmd

    import concourse.mybir as mybir

    nc = _get_nc()
    mm_np = mybir.dt.np(getattr(mybir.dt, MM_DTYPE))
    s3_np = mybir.dt.np(getattr(mybir.dt, S3_DTYPE))

    x = np.concatenate(
        [
            np.asarray(t, dtype=np.float32).reshape(B_TOTAL, -1)
            for t in (result_ls, result_A, result_lm, result_AT, result_ds, result_dm)
        ],
        axis=1,
    )  # [8192, 4096]
    W = np.asarray(W, dtype=np.float32)
    b = np.asarray(b, dtype=np.float32)
    h_n = np.asarray(h_n, dtype=np.float32)

    wT = np.ascontiguousarray(W[:, :DW].T).astype(mm_np)       # [1024, 512]
    wt = wT.reshape(2, 4, 128, OUT).transpose(0, 2, 1, 3)      # [2, 128, 4, 512]
    wt = np.ascontiguousarray(wt)
    bias = np.ascontiguousarray(b.reshape(4, 128).T)           # [128, 4]
    # h6[p, n*4 + oc, c] = h[oc*128 + p] if c == n else 0
    h6 = np.zeros((128, 24, 128), dtype=np.float32)
    for n in range(NODES):
        for oc in range(4):
            h6[:, n * 4 + oc, n] = h_n[oc * 128 : (oc + 1) * 128, 0]
    h6 = h6.astype(mm_np)
    eye = np.eye(6, dtype=np.float32)
    eye128 = np.eye(128, dtype=s3_np)

    in_maps = []
    for c in range(NCORES):
        xc = x[c * BLOC : (c + 1) * BLOC]                      # [1024, 4096]
        xT = xc.T                                              # [4096, 1024]
        xt = np.empty((NBC, NG, 128, GK, BC), dtype=mm_np)
        for bc in range(NBC):
            blk = xT[:, bc * BC : (bc + 1) * BC].reshape(NG, GK, 128, BC)
            xt[bc] = blk.transpose(0, 2, 1, 3)
        in_maps.append(
            {
                "xt": xt,
                "xb": np.ascontiguousarray(xc.reshape(8, 128, 4096)).astype(s3_np),
                "wt": wt.reshape(2, 128, 4 * OUT),
                "bias": bias,
                "h6": h6,
                "eye": eye,
                "eye128": eye128,
            }
        )

    res = run_bass_kernel_spmd(nc, in_maps, list(range(NCORES)))
    LAST_RESULT = res
    LAST_EXEC_TIME_NS = res.exec_time_ns

    out = np.zeros((B_TOTAL, 1, 2048), dtype=np.float32)
    for c in range(NCORES):
        zc = res.results[c]["z"]                               # [8, 128, 1024]
        out[c * BLOC : (c + 1) * BLOC, 0, :DW] = zc.reshape(BLOC, DW).astype(np.float32)
    return out


# revision 14
# speedup vs baseline: 1.1804x; 1.1804x over previous
"""Trainium2 Bass kernel for nn_Beta_score2 (gnn_message_passing).

Computation (per batch element b):
  nodes   = 6 feature vectors x_k (padded to 2048; padding never contributes)
  temp_k  = tanh(x_k @ W[:, :d_k]^T + b)          # [512]
  score_k = temp_k . h_n                           # scalar
  beta    = softmax(score)                         # [6]
  z       = sum_k beta_k * pad(x_k)                # [2048], cols 1024: always 0

Sharding: data-parallel over batch, B=8192 -> 1024 per core on 8 cores.

Per-core pipeline (two 512-wide batch chunks):
  stage 1: PE matmuls W^T-chunks x xT-chunks -> PSUM [128o, 512b];
           ACT fused bias+tanh -> temp^T in SBUF (bf16; PE runs bf16 at the
           full 2.4 GHz while fp16 is throttled ~20%).
  stage 2: score matmuls use 128-col zero-padded h-tiles (enables FWL fast
           weight load -> big-matmul speed) accumulating all 24 (node, oc)
           pieces into one PSUM [128, 512] whose rows 0:6 are the scores;
           PE-transpose to batch-major, softmax on ACT/DVE -> beta [128b, 24].
  stage 3: batch-major weighted sum with beta as per-partition scalars:
           ACT z = copy(x0 * b0), DVE scalar_tensor_tensor z += bk * xk.
           x for this stage (xb, fp16) is the natural row-major layout so its
           DMA is fully contiguous; z leaves batch-major (no host transpose).
  head: PE warm-up junk matmuls run during the initial DMA wait so the HAM
        clock gate releases before real matmuls; chunk-0 stage-2/3 emission
        is deferred past chunk-1's first node so the PE queue never blocks
        on the softmax transposes.

Host pre-tiles every DRAM tensor so each dma_start is a contiguous copy.
"""

import os
import sys
import types

import numpy as np

B_TOTAL = 8192
NCORES = 8
BLOC = B_TOTAL // NCORES  # 1024
OUT = 512
DW = 1024                 # only W[:, :1024] is ever used
NODES = 6
NODE_OFF = [0, 1024, 1536, 2048, 2560, 3584]
NODE_DIM = [1024, 512, 512, 512, 1024, 512]
NODE_ORDER = (1, 0, 4, 2, 3, 5)   # 1 first (single x-group), then 8-kc nodes
                                  # so the tanh pipeline gets slack
BC = 512                  # batch chunk on the free dim
NBC = BLOC // BC
GK = 4                    # xT group: [128, GK, BC]
NG = 8                    # 32 kc-chunks / GK

MM_DTYPE = os.environ.get("KERNEL_MM_DTYPE", "bfloat16")
S3_DTYPE = os.environ.get("KERNEL_S3_DTYPE", "float16")
N_JUNK = int(os.environ.get("KERNEL_N_JUNK", "11"))

LAST_EXEC_TIME_NS = None
LAST_RESULT = None

_cache = {}


def _install_ntff_hook():
    """run_bass_kernel_spmd(trace=True) under axon needs antenv.axon_hooks,
    which this image lacks; synthesize it from trn_agent_boot."""
    if "antenv.axon_hooks" in sys.modules:
        return
    try:
        import antenv
        import trn_agent_boot.trn_boot as tb
    except Exception:
        return
    mod = types.ModuleType("antenv.axon_hooks")
    _hook = tb._ntff_profile_via_ctypes("/opt/axon/libaxon_pjrt.so")
    mod.get_axon_ntff_profile_hook = lambda: _hook
    mod.set_axon_ntff_profile_hook = lambda h: None
    sys.modules["antenv.axon_hooks"] = mod
    antenv.axon_hooks = mod


def _build(mm_dtype_name, s3_dtype_name):
    from contextlib import ExitStack

    import concourse.bacc as bacc
    import concourse.mybir as mybir
    import concourse.tile as tile

    f32 = mybir.dt.float32
    mm_dt = getattr(mybir.dt, mm_dtype_name)
    s3_dt = getattr(mybir.dt, s3_dtype_name)

    nc = bacc.Bacc("TRN2", target_bir_lowering=False, debug=False)
    # pre-tiled inputs: every dma_start below is a contiguous copy
    xt_d = nc.dram_tensor("xt", [NBC, NG, 128, GK, BC], mm_dt, kind="ExternalInput").ap()
    xb_d = nc.dram_tensor("xb", [8, 128, 4096], s3_dt, kind="ExternalInput").ap()
    wt_d = nc.dram_tensor("wt", [2, 128, 4 * OUT], mm_dt, kind="ExternalInput").ap()
    bias_d = nc.dram_tensor("bias", [128, 4], f32, kind="ExternalInput").ap()
    h6_d = nc.dram_tensor("h6", [128, 24, 128], mm_dt, kind="ExternalInput").ap()
    eye_d = nc.dram_tensor("eye", [6, 6], f32, kind="ExternalInput").ap()
    eye128_d = nc.dram_tensor("eye128", [128, 128], s3_dt, kind="ExternalInput").ap()
    z_d = nc.dram_tensor("z", [8, 128, DW], s3_dt, kind="ExternalOutput").ap()

    Tanh = mybir.ActivationFunctionType.Tanh
    Exp = mybir.ActivationFunctionType.Exp
    Copy = mybir.ActivationFunctionType.Copy
    Mult = mybir.AluOpType.mult
    Add = mybir.AluOpType.add

    with tile.TileContext(nc) as tc, ExitStack() as ctx:
        const = ctx.enter_context(tc.tile_pool(name="const", bufs=1))
        wt_lo = const.tile([128, 4, OUT], mm_dt)
        wt_hi = const.tile([128, 4, OUT], mm_dt)
        bias_t = const.tile([128, 4], f32)
        h6_t = const.tile([128, 24, 128], mm_dt)
        eye_t = const.tile([6, 6], f32)
        eye128_t = const.tile([128, 128], s3_dt)
        junk_w = const.tile([128, 128], mm_dt, name="junk_w")
        junk_x = const.tile([128, BC], mm_dt, name="junk_x")

        pre_ps = ctx.enter_context(tc.tile_pool(name="pre", bufs=4, space="PSUM"))
        score_ps = ctx.enter_context(tc.tile_pool(name="score", bufs=2, space="PSUM"))
        tp_ps = ctx.enter_context(tc.tile_pool(name="tp", bufs=1, space="PSUM"))
        temps = ctx.enter_context(tc.tile_pool(name="temps", bufs=4))
        small = ctx.enter_context(tc.tile_pool(name="small", bufs=2))
        zpool = ctx.enter_context(tc.tile_pool(name="zpool", bufs=3))
        diag_pool = ctx.enter_context(tc.tile_pool(name="diag", bufs=12))

        # ---- PE warm-up: junk matmuls release the HAM clock gate while the
        # first DMAs are in flight.  memsets on GpSimd (idle at start).
        nc.gpsimd.memset(junk_w[:], 0.0)
        nc.gpsimd.memset(junk_x[:], 0.0)
        jp = score_ps.tile([128, BC], f32, name="junk_ps", tag="sc")
        for i in range(N_JUNK):
            nc.tensor.matmul(jp[:], junk_w[:], junk_x[:], start=True, stop=True)
        # dummy tanh pulls the 1.3us ACT table load into the DMA-wait window
        warm_t = temps.tile([128, BC], mm_dt, tag="tt", name="warm_t")
        nc.scalar.activation(warm_t[:], junk_x[:], Tanh, bias=0.0, scale=1.0)

        # ---- DMA dispatch. Critical path on Sync; bulk x streams on GpSimd.
        xts_tiles = {}

        def load_xt(bc, g, eng):
            t = const.tile([128, GK, BC], mm_dt, name=f"xt_{bc}_{g}")
            eng.dma_start(t[:], xt_d[bc, g])
            xts_tiles[(g, bc)] = t

        xb_tiles = {}

        def load_xb(j, eng):
            t = const.tile([128, 4096], s3_dt, name=f"xb_{j}")
            eng.dma_start(t[:], xb_d[j])
            xb_tiles[j] = t

        # critical path on the Sync queue only (MM#1's semaphore wait is
        # batched per-queue, so nothing else may ride on it); h6/eyes go on
        # the Scalar queue, bulk x on GpSimd.
        nc.sync.dma_start(wt_lo[:], wt_d[0])
        load_xt(0, 2, nc.sync)
        nc.sync.dma_start(bias_t[:], bias_d[:, :])
        nc.scalar.dma_start(wt_hi[:], wt_d[1])
        nc.scalar.dma_start(h6_t[:], h6_d[:, :, :])
        nc.scalar.dma_start(eye_t[:], eye_d[:, :])
        nc.scalar.dma_start(eye128_t[:], eye128_d[:, :])
        for g in (0, 1, 5, 6, 3, 4, 7):
            load_xt(0, g, nc.gpsimd)
        for g in (2, 0, 1):
            load_xt(1, g, nc.gpsimd)
        for j in range(4):
            load_xb(j, nc.gpsimd)
        for g in (5, 6, 3, 4, 7):
            load_xt(1, g, nc.gpsimd)
        for j in range(4, 8):
            load_xb(j, nc.gpsimd)

        def xts(kc, bc):
            return xts_tiles[(kc // GK, bc)][:, kc % GK, :]

        def wts(kc, oc):
            w = wt_lo if kc < 4 else wt_hi
            return w[:, kc % 4, oc * 128 : (oc + 1) * 128]

        # Score matmuls are emitted one group late ("pending"), so the PE
        # always has the next group's main matmuls queued between a tanh and
        # the score matmul that consumes it.
        pending_sc = []

        def flush_sc():
            if pending_sc:
                sc_t, lhsT, rhs, st, sp = pending_sc.pop()
                nc.tensor.matmul(sc_t[:], lhsT, rhs, start=st, stop=sp)

        scs = {}

        def emit_stage1_node(bc, n):
            if n == NODE_ORDER[0]:
                scs[bc] = score_ps.tile([128, BC], f32, name=f"sc_{bc}", tag="sc")
            sc = scs[bc]
            nk = NODE_DIM[n] // 128
            off = NODE_OFF[n] // 128
            for oc in range(4):
                ps = pre_ps.tile([128, BC], f32)
                for kc in range(nk):
                    nc.tensor.matmul(
                        ps[:],
                        wts(kc, oc),
                        xts(off + kc, bc),
                        start=(kc == 0),
                        stop=(kc == nk - 1),
                    )
                tt = temps.tile([128, BC], mm_dt, tag="tt", name="tt")
                nc.scalar.activation(
                    tt[:], ps[:], Tanh, bias=bias_t[:, oc : oc + 1], scale=1.0
                )
                flush_sc()
                pending_sc.append(
                    (
                        sc,
                        h6_t[:, n * 4 + oc, :],
                        tt[:],
                        (n == NODE_ORDER[0] and oc == 0),
                        (n == NODE_ORDER[-1] and oc == 3),
                    )
                )

        def emit_stage23(bc):
            sc = scs[bc]
            # ---------- softmax over the 6 nodes (batch-major) ----------
            sc_sb = small.tile([6, BC], f32, tag="sc_sb")
            nc.scalar.copy(sc_sb[:], sc[0:6, :])
            tp = tp_ps.tile([128, 24], f32)
            for j in range(4):
                nc.tensor.transpose(
                    tp[:, j * 6 : (j + 1) * 6],
                    sc_sb[:, j * 128 : (j + 1) * 128],
                    eye_t[:],
                )
            expt = small.tile([128, 24], f32, tag="expt")
            sumexp = small.tile([128, 4], f32, tag="sumexp")
            nc.scalar.activation(expt[:], tp[:], Exp)
            nc.vector.tensor_reduce(
                sumexp[:],
                expt[:].rearrange("p (j k) -> p j k", j=4),
                axis=mybir.AxisListType.X,
                op=mybir.AluOpType.add,
            )
            rec = small.tile([128, 4], f32, tag="rec")
            nc.vector.reciprocal(rec[:], sumexp[:])
            beta = small.tile([128, 24], f32, tag="beta")
            for j in range(4):
                nc.vector.tensor_scalar_mul(
                    beta[:, j * 6 : (j + 1) * 6],
                    expt[:, j * 6 : (j + 1) * 6],
                    rec[:, j : j + 1],
                )
            # ---------- stage 3: batch-major z = sum_k beta_k * x_k ----------
            def dve_group(j):
                # ACT z-init + DVE FMA chain
                xb = xb_tiles[bc * 4 + j]
                bf = beta[:, j * 6 : j * 6 + 6]
                z = zpool.tile([128, DW], s3_dt, tag="z", name=f"z_{bc}_{j}")
                nc.scalar.activation(z[:], xb[:, 0:DW], Copy, scale=bf[:, 0:1])
                nc.vector.scalar_tensor_tensor(
                    z[:], xb[:, 2560:3584], bf[:, 4:5], z[:], Mult, Add
                )
                for k, lo in ((1, 1024), (2, 1536), (3, 2048), (5, 3584)):
                    nc.vector.scalar_tensor_tensor(
                        z[:, 0:512], xb[:, lo : lo + 512], bf[:, k : k + 1],
                        z[:, 0:512], Mult, Add,
                    )
                nc.sync.dma_start(z_d[bc * 4 + j], z[:])

            if bc < NBC - 1:
                for j in range(4):
                    dve_group(j)
            else:
                # tail chunk: j3 on ACT+DVE; j0/j1/j2 as PE diag-matmuls
                # (diag tiles for j0 built on DVE, j1/j2 on ACT); copies
                # back from PSUM split ACT (j0, j2) / DVE (j1).
                dve_group(3)

                def build_diags(j, eng):
                    bf = beta[:, j * 6 : j * 6 + 6]
                    diags = []
                    for k in range(6):
                        d = diag_pool.tile([128, 128], s3_dt, tag="dg", name=f"dg{j}_{k}")
                        if eng == "dve":
                            nc.vector.tensor_scalar_mul(
                                d[:], eye128_t[:], bf[:, k : k + 1]
                            )
                        else:
                            nc.scalar.activation(
                                d[:], eye128_t[:], Copy, scale=bf[:, k : k + 1]
                            )
                        diags.append(d)
                    return diags

                def pe_group_mm(j, diags):
                    xb = xb_tiles[bc * 4 + j]
                    za = pre_ps.tile([128, BC], f32, tag="ps", name=f"za_{j}")
                    for i, (k, lo) in enumerate(
                        ((0, 0), (1, 1024), (2, 1536), (3, 2048), (4, 2560), (5, 3584))
                    ):
                        nc.tensor.matmul(
                            za[:], diags[k][:], xb[:, lo : lo + 512],
                            start=(i == 0), stop=(i == 5),
                        )
                    zb = pre_ps.tile([128, BC], f32, tag="ps", name=f"zb_{j}")
                    nc.tensor.matmul(
                        zb[:], diags[0][:], xb[:, 512:1024], start=True, stop=False
                    )
                    nc.tensor.matmul(
                        zb[:], diags[4][:], xb[:, 3072:3584], start=False, stop=True
                    )
                    return za, zb

                def pe_group_out(j, za, zb, eng):
                    z = zpool.tile([128, DW], s3_dt, tag="z", name=f"zd_{j}")
                    if eng == "dve":
                        nc.vector.tensor_copy(z[:, 0:512], za[:])
                        nc.vector.tensor_copy(z[:, 512:1024], zb[:])
                    else:
                        nc.scalar.copy(z[:, 0:512], za[:])
                        nc.scalar.copy(z[:, 512:1024], zb[:])
                    nc.sync.dma_start(z_d[bc * 4 + j], z[:])

                d0 = build_diags(0, "dve")
                za0, zb0 = pe_group_mm(0, d0)
                d1 = build_diags(1, "act")
                za1, zb1 = pe_group_mm(1, d1)
                d2 = build_diags(2, "act")
                za2, zb2 = pe_group_mm(2, d2)
                pe_group_out(0, za0, zb0, "act")
                pe_group_out(1, za1, zb1, "dve")
                pe_group_out(2, za2, zb2, "act")

        # chunk 0 stage 1; defer its stage 2/3 past chunk 1's first node so
        # the softmax transposes never block the PE queue.
        for n in NODE_ORDER:
            emit_stage1_node(0, n)
        emit_stage1_node(1, NODE_ORDER[0])
        emit_stage23(0)
        for n in NODE_ORDER[1:]:
            emit_stage1_node(1, n)
        flush_sc()
        # keep the PE busy through the softmax window so the HAM clock gate
        # stays open for the tail diag-matmuls
        jp2 = pre_ps.tile([128, BC], f32, tag="ps", name="jp2")
        for i in range(10):
            nc.tensor.matmul(jp2[:], junk_w[:], junk_x[:], start=True, stop=True)
        emit_stage23(1)

    nc.compile()
    return nc


def _get_nc():
    key = (MM_DTYPE, S3_DTYPE)
    if key not in _cache:
        _cache[key] = _build(*key)
    return _cache[key]


def kernel(result_ls, result_A, result_lm, result_AT, result_ds, result_dm, W, b, h_n):
    global LAST_EXEC_TIME_NS, LAST_RESULT
    _install_ntff_hook()
    from concourse.bass_utils import run_bass_kernel_spmd

    import concourse.mybir as mybir

    nc = _get_nc()
    mm_np = mybir.dt.np(getattr(mybir.dt, MM_DTYPE))
    s3_np = mybir.dt.np(getattr(mybir.dt, S3_DTYPE))

    x = np.concatenate(
        [
            np.asarray(t, dtype=np.float32).reshape(B_TOTAL, -1)
            for t in (result_ls, result_A, result_lm, result_AT, result_ds, result_dm)
        ],
        axis=1,
    )  # [8192, 4096]
    W = np.asarray(W, dtype=np.float32)
    b = np.asarray(b, dtype=np.float32)
    h_n = np.asarray(h_n, dtype=np.float32)

    wT = np.ascontiguousarray(W[:, :DW].T).astype(mm_np)       # [1024, 512]
    wt = wT.reshape(2, 4, 128, OUT).transpose(0, 2, 1, 3)      # [2, 128, 4, 512]
    wt = np.ascontiguousarray(wt)
    bias = np.ascontiguousarray(b.reshape(4, 128).T)           # [128, 4]
    # h6[p, n*4 + oc, c] = h[oc*128 + p] if c == n else 0
    h6 = np.zeros((128, 24, 128), dtype=np.float32)
    for n in range(NODES):
        for oc in range(4):
            h6[:, n * 4 + oc, n] = h_n[oc * 128 : (oc + 1) * 128, 0]
    h6 = h6.astype(mm_np)
    eye = np.eye(6, dtype=np.float32)
    eye128 = np.eye(128, dtype=s3_np)

    in_maps = []
    for c in range(NCORES):
        xc = x[c * BLOC : (c + 1) * BLOC]                      # [1024, 4096]
        xT = xc.T                                              # [4096, 1024]
        xt = np.empty((NBC, NG, 128, GK, BC), dtype=mm_np)
        for bc in range(NBC):
            blk = xT[:, bc * BC : (bc + 1) * BC].reshape(NG, GK, 128, BC)
            xt[bc] = blk.transpose(0, 2, 1, 3)
        in_maps.append(
            {
                "xt": xt,
                "xb": np.ascontiguousarray(xc.reshape(8, 128, 4096)).astype(s3_np),
                "wt": wt.reshape(2, 128, 4 * OUT),
                "bias": bias,
                "h6": h6,
                "eye": eye,
                "eye128": eye128,
            }
        )

    res = run_bass_kernel_spmd(nc, in_maps, list(range(NCORES)))
    LAST_RESULT = res
    LAST_EXEC_TIME_NS = res.exec_time_ns

    out = np.zeros((B_TOTAL, 1, 2048), dtype=np.float32)
    for c in range(NCORES):
        zc = res.results[c]["z"]                               # [8, 128, 1024]
        out[c * BLOC : (c + 1) * BLOC, 0, :DW] = zc.reshape(BLOC, DW).astype(np.float32)
    return out


# revision 15
# speedup vs baseline: 1.1831x; 1.0023x over previous
"""Trainium2 Bass kernel for nn_Beta_score2 (gnn_message_passing).

Computation (per batch element b):
  nodes   = 6 feature vectors x_k (padded to 2048; padding never contributes)
  temp_k  = tanh(x_k @ W[:, :d_k]^T + b)          # [512]
  score_k = temp_k . h_n                           # scalar
  beta    = softmax(score)                         # [6]
  z       = sum_k beta_k * pad(x_k)                # [2048], cols 1024: always 0

Sharding: data-parallel over batch, B=8192 -> 1024 per core on 8 cores.

Per-core pipeline (two 512-wide batch chunks):
  stage 1: PE matmuls W^T-chunks x xT-chunks -> PSUM [128o, 512b];
           ACT fused bias+tanh -> temp^T in SBUF (bf16; PE runs bf16 at the
           full 2.4 GHz while fp16 is throttled ~20%).
  stage 2: score matmuls use 128-col zero-padded h-tiles (enables FWL fast
           weight load -> big-matmul speed) accumulating all 24 (node, oc)
           pieces into one PSUM [128, 512] whose rows 0:6 are the scores;
           PE-transpose to batch-major, softmax on ACT/DVE -> beta [128b, 24].
  stage 3: batch-major weighted sum with beta as per-partition scalars:
           ACT z = copy(x0 * b0), DVE scalar_tensor_tensor z += bk * xk.
           x for this stage (xb, fp16) is the natural row-major layout so its
           DMA is fully contiguous; z leaves batch-major (no host transpose).
  head: PE warm-up junk matmuls run during the initial DMA wait so the HAM
        clock gate releases before real matmuls; chunk-0 stage-2/3 emission
        is deferred past chunk-1's first node so the PE queue never blocks
        on the softmax transposes.

Host pre-tiles every DRAM tensor so each dma_start is a contiguous copy.
"""

import os
import sys
import types

import numpy as np

B_TOTAL = 8192
NCORES = 8
BLOC = B_TOTAL // NCORES  # 1024
OUT = 512
DW = 1024                 # only W[:, :1024] is ever used
NODES = 6
NODE_OFF = [0, 1024, 1536, 2048, 2560, 3584]
NODE_DIM = [1024, 512, 512, 512, 1024, 512]
NODE_ORDER = (1, 0, 4, 2, 3, 5)   # 1 first (single x-group), then 8-kc nodes
                                  # so the tanh pipeline gets slack
BC = 512                  # batch chunk on the free dim
NBC = BLOC // BC
GK = 4                    # xT group: [128, GK, BC]
NG = 8                    # 32 kc-chunks / GK

MM_DTYPE = os.environ.get("KERNEL_MM_DTYPE", "bfloat16")
S3_DTYPE = os.environ.get("KERNEL_S3_DTYPE", "float16")
N_JUNK = int(os.environ.get("KERNEL_N_JUNK", "11"))

LAST_EXEC_TIME_NS = None
LAST_RESULT = None

_cache = {}


def _install_ntff_hook():
    """run_bass_kernel_spmd(trace=True) under axon needs antenv.axon_hooks,
    which this image lacks; synthesize it from trn_agent_boot."""
    if "antenv.axon_hooks" in sys.modules:
        return
    try:
        import antenv
        import trn_agent_boot.trn_boot as tb
    except Exception:
        return
    mod = types.ModuleType("antenv.axon_hooks")
    _hook = tb._ntff_profile_via_ctypes("/opt/axon/libaxon_pjrt.so")
    mod.get_axon_ntff_profile_hook = lambda: _hook
    mod.set_axon_ntff_profile_hook = lambda h: None
    sys.modules["antenv.axon_hooks"] = mod
    antenv.axon_hooks = mod


def _build(mm_dtype_name, s3_dtype_name):
    from contextlib import ExitStack

    import concourse.bacc as bacc
    import concourse.mybir as mybir
    import concourse.tile as tile

    f32 = mybir.dt.float32
    mm_dt = getattr(mybir.dt, mm_dtype_name)
    s3_dt = getattr(mybir.dt, s3_dtype_name)

    nc = bacc.Bacc("TRN2", target_bir_lowering=False, debug=False)
    # pre-tiled inputs: every dma_start below is a contiguous copy
    xt_d = nc.dram_tensor("xt", [NBC, NG, 128, GK, BC], mm_dt, kind="ExternalInput").ap()
    xb_d = nc.dram_tensor("xb", [8, 128, 4096], s3_dt, kind="ExternalInput").ap()
    wt_d = nc.dram_tensor("wt", [2, 128, 4 * OUT], mm_dt, kind="ExternalInput").ap()
    bias_d = nc.dram_tensor("bias", [128, 4], f32, kind="ExternalInput").ap()
    h6_d = nc.dram_tensor("h6", [128, 24, 128], mm_dt, kind="ExternalInput").ap()
    eye_d = nc.dram_tensor("eye", [6, 6], f32, kind="ExternalInput").ap()
    eye128_d = nc.dram_tensor("eye128", [128, 128], s3_dt, kind="ExternalInput").ap()
    z_d = nc.dram_tensor("z", [8, 128, DW], s3_dt, kind="ExternalOutput").ap()

    Tanh = mybir.ActivationFunctionType.Tanh
    Exp = mybir.ActivationFunctionType.Exp
    Copy = mybir.ActivationFunctionType.Copy
    Mult = mybir.AluOpType.mult
    Add = mybir.AluOpType.add

    with tile.TileContext(nc) as tc, ExitStack() as ctx:
        const = ctx.enter_context(tc.tile_pool(name="const", bufs=1))
        wt_lo = const.tile([128, 4, OUT], mm_dt)
        wt_hi = const.tile([128, 4, OUT], mm_dt)
        bias_t = const.tile([128, 4], f32)
        h6_t = const.tile([128, 24, 128], mm_dt)
        eye_t = const.tile([6, 6], f32)
        eye128_t = const.tile([128, 128], s3_dt)
        junk_w = const.tile([128, 128], mm_dt, name="junk_w")
        junk_x = const.tile([128, BC], mm_dt, name="junk_x")

        pre_ps = ctx.enter_context(tc.tile_pool(name="pre", bufs=4, space="PSUM"))
        score_ps = ctx.enter_context(tc.tile_pool(name="score", bufs=2, space="PSUM"))
        tp_ps = ctx.enter_context(tc.tile_pool(name="tp", bufs=1, space="PSUM"))
        temps = ctx.enter_context(tc.tile_pool(name="temps", bufs=4))
        small = ctx.enter_context(tc.tile_pool(name="small", bufs=2))
        zpool = ctx.enter_context(tc.tile_pool(name="zpool", bufs=3))
        diag_pool = ctx.enter_context(tc.tile_pool(name="diag", bufs=12))

        # ---- PE warm-up: junk matmuls release the HAM clock gate while the
        # first DMAs are in flight.  memsets on GpSimd (idle at start).
        nc.gpsimd.memset(junk_w[:], 0.0)
        nc.gpsimd.memset(junk_x[:], 0.0)
        jp_a = pre_ps.tile([128, BC], f32, name="jp_a", tag="ps")
        jp = score_ps.tile([128, BC], f32, name="junk_ps", tag="sc")
        for i in range(4):
            nc.tensor.matmul(jp_a[:], junk_w[:], junk_x[:], start=True, stop=True)
        for i in range(N_JUNK - 4):
            nc.tensor.matmul(jp[:], junk_w[:], junk_x[:], start=True, stop=True)
        # dummy tanh pulls the 1.3us ACT table load into the DMA-wait window
        warm_t = temps.tile([128, BC], mm_dt, tag="tt", name="warm_t")
        nc.scalar.activation(warm_t[:], junk_x[:], Tanh, bias=0.0, scale=1.0)

        # ---- DMA dispatch. Critical path on Sync; bulk x streams on GpSimd.
        xts_tiles = {}

        def load_xt(bc, g, eng):
            t = const.tile([128, GK, BC], mm_dt, name=f"xt_{bc}_{g}")
            eng.dma_start(t[:], xt_d[bc, g])
            xts_tiles[(g, bc)] = t

        xb_tiles = {}

        def load_xb(j, eng):
            t = const.tile([128, 4096], s3_dt, name=f"xb_{j}")
            eng.dma_start(t[:], xb_d[j])
            xb_tiles[j] = t

        # Critical tiles ride alone on the Sync queue (MM#1's semaphore wait
        # is batched per-queue) and get the DMA bus to themselves until the
        # 4th junk matmul lifts the gate; everything else streams on GpSimd
        # in exact consumption order.
        nc.sync.dma_start(wt_lo[:], wt_d[0])
        load_xt(0, 2, nc.sync)
        nc.sync.dma_start(bias_t[:], bias_d[:, :])
        gate1 = const.tile([1, 1], f32, name="gate1")
        gate2 = const.tile([1, 1], f32, name="gate2")
        nc.vector.tensor_copy(gate1[:], jp_a[0:1, 0:1])
        nc.gpsimd.tensor_copy(gate2[:], gate1[:])
        nc.gpsimd.dma_start(wt_hi[:], wt_d[1])
        nc.gpsimd.dma_start(h6_t[:], h6_d[:, :, :])
        for g in (0, 1, 5, 6, 3, 4, 7):
            load_xt(0, g, nc.gpsimd)
        nc.gpsimd.dma_start(eye_t[:], eye_d[:, :])
        nc.gpsimd.dma_start(eye128_t[:], eye128_d[:, :])
        for g in (2, 0, 1):
            load_xt(1, g, nc.gpsimd)
        for j in range(4):
            load_xb(j, nc.gpsimd)
        for g in (5, 6, 3, 4, 7):
            load_xt(1, g, nc.gpsimd)
        for j in range(4, 8):
            load_xb(j, nc.gpsimd)

        def xts(kc, bc):
            return xts_tiles[(kc // GK, bc)][:, kc % GK, :]

        def wts(kc, oc):
            w = wt_lo if kc < 4 else wt_hi
            return w[:, kc % 4, oc * 128 : (oc + 1) * 128]

        # Score matmuls are emitted one group late ("pending"), so the PE
        # always has the next group's main matmuls queued between a tanh and
        # the score matmul that consumes it.
        pending_sc = []

        def flush_sc():
            if pending_sc:
                sc_t, lhsT, rhs, st, sp = pending_sc.pop()
                nc.tensor.matmul(sc_t[:], lhsT, rhs, start=st, stop=sp)

        scs = {}

        def emit_stage1_node(bc, n):
            if n == NODE_ORDER[0]:
                scs[bc] = score_ps.tile([128, BC], f32, name=f"sc_{bc}", tag="sc")
            sc = scs[bc]
            nk = NODE_DIM[n] // 128
            off = NODE_OFF[n] // 128
            for oc in range(4):
                ps = pre_ps.tile([128, BC], f32)
                for kc in range(nk):
                    nc.tensor.matmul(
                        ps[:],
                        wts(kc, oc),
                        xts(off + kc, bc),
                        start=(kc == 0),
                        stop=(kc == nk - 1),
                    )
                tt = temps.tile([128, BC], mm_dt, tag="tt", name="tt")
                nc.scalar.activation(
                    tt[:], ps[:], Tanh, bias=bias_t[:, oc : oc + 1], scale=1.0
                )
                flush_sc()
                pending_sc.append(
                    (
                        sc,
                        h6_t[:, n * 4 + oc, :],
                        tt[:],
                        (n == NODE_ORDER[0] and oc == 0),
                        (n == NODE_ORDER[-1] and oc == 3),
                    )
                )

        def emit_stage23(bc):
            sc = scs[bc]
            # ---------- softmax over the 6 nodes (batch-major) ----------
            sc_sb = small.tile([6, BC], f32, tag="sc_sb")
            nc.scalar.copy(sc_sb[:], sc[0:6, :])
            tp = tp_ps.tile([128, 24], f32)
            for j in range(4):
                nc.tensor.transpose(
                    tp[:, j * 6 : (j + 1) * 6],
                    sc_sb[:, j * 128 : (j + 1) * 128],
                    eye_t[:],
                )
            expt = small.tile([128, 24], f32, tag="expt")
            sumexp = small.tile([128, 4], f32, tag="sumexp")
            nc.scalar.activation(expt[:], tp[:], Exp)
            nc.vector.tensor_reduce(
                sumexp[:],
                expt[:].rearrange("p (j k) -> p j k", j=4),
                axis=mybir.AxisListType.X,
                op=mybir.AluOpType.add,
            )
            rec = small.tile([128, 4], f32, tag="rec")
            nc.vector.reciprocal(rec[:], sumexp[:])
            beta = small.tile([128, 24], f32, tag="beta")
            for j in range(4):
                nc.vector.tensor_scalar_mul(
                    beta[:, j * 6 : (j + 1) * 6],
                    expt[:, j * 6 : (j + 1) * 6],
                    rec[:, j : j + 1],
                )
            # ---------- stage 3: batch-major z = sum_k beta_k * x_k ----------
            def dve_group(j):
                # ACT z-init + DVE FMA chain
                xb = xb_tiles[bc * 4 + j]
                bf = beta[:, j * 6 : j * 6 + 6]
                z = zpool.tile([128, DW], s3_dt, tag="z", name=f"z_{bc}_{j}")
                nc.scalar.activation(z[:], xb[:, 0:DW], Copy, scale=bf[:, 0:1])
                nc.vector.scalar_tensor_tensor(
                    z[:], xb[:, 2560:3584], bf[:, 4:5], z[:], Mult, Add
                )
                for k, lo in ((1, 1024), (2, 1536), (3, 2048), (5, 3584)):
                    nc.vector.scalar_tensor_tensor(
                        z[:, 0:512], xb[:, lo : lo + 512], bf[:, k : k + 1],
                        z[:, 0:512], Mult, Add,
                    )
                nc.sync.dma_start(z_d[bc * 4 + j], z[:])

            if bc < NBC - 1:
                for j in range(4):
                    dve_group(j)
            else:
                # tail chunk: j3 on ACT+DVE; j0/j1/j2 as PE diag-matmuls
                # (diag tiles for j0 built on DVE, j1/j2 on ACT); copies
                # back from PSUM split ACT (j0, j2) / DVE (j1).
                dve_group(3)

                def build_diags(j, eng):
                    bf = beta[:, j * 6 : j * 6 + 6]
                    diags = []
                    for k in range(6):
                        d = diag_pool.tile([128, 128], s3_dt, tag="dg", name=f"dg{j}_{k}")
                        if eng == "dve":
                            nc.vector.tensor_scalar_mul(
                                d[:], eye128_t[:], bf[:, k : k + 1]
                            )
                        else:
                            nc.scalar.activation(
                                d[:], eye128_t[:], Copy, scale=bf[:, k : k + 1]
                            )
                        diags.append(d)
                    return diags

                def pe_group_mm(j, diags):
                    xb = xb_tiles[bc * 4 + j]
                    za = pre_ps.tile([128, BC], f32, tag="ps", name=f"za_{j}")
                    for i, (k, lo) in enumerate(
                        ((0, 0), (1, 1024), (2, 1536), (3, 2048), (4, 2560), (5, 3584))
                    ):
                        nc.tensor.matmul(
                            za[:], diags[k][:], xb[:, lo : lo + 512],
                            start=(i == 0), stop=(i == 5),
                        )
                    zb = pre_ps.tile([128, BC], f32, tag="ps", name=f"zb_{j}")
                    nc.tensor.matmul(
                        zb[:], diags[0][:], xb[:, 512:1024], start=True, stop=False
                    )
                    nc.tensor.matmul(
                        zb[:], diags[4][:], xb[:, 3072:3584], start=False, stop=True
                    )
                    return za, zb

                def pe_group_out(j, za, zb, eng):
                    z = zpool.tile([128, DW], s3_dt, tag="z", name=f"zd_{j}")
                    if eng == "dve":
                        nc.vector.tensor_copy(z[:, 0:512], za[:])
                        nc.vector.tensor_copy(z[:, 512:1024], zb[:])
                    else:
                        nc.scalar.copy(z[:, 0:512], za[:])
                        nc.scalar.copy(z[:, 512:1024], zb[:])
                    nc.sync.dma_start(z_d[bc * 4 + j], z[:])

                d0 = build_diags(0, "dve")
                za0, zb0 = pe_group_mm(0, d0)
                d1 = build_diags(1, "act")
                za1, zb1 = pe_group_mm(1, d1)
                d2 = build_diags(2, "act")
                za2, zb2 = pe_group_mm(2, d2)
                pe_group_out(0, za0, zb0, "act")
                pe_group_out(1, za1, zb1, "dve")
                pe_group_out(2, za2, zb2, "act")

        # chunk 0 stage 1; defer its stage 2/3 past chunk 1's first node so
        # the softmax transposes never block the PE queue.
        for n in NODE_ORDER:
            emit_stage1_node(0, n)
        emit_stage1_node(1, NODE_ORDER[0])
        emit_stage23(0)
        for n in NODE_ORDER[1:]:
            emit_stage1_node(1, n)
        flush_sc()
        # keep the PE busy through the softmax window so the HAM clock gate
        # stays open for the tail diag-matmuls
        jp2 = pre_ps.tile([128, BC], f32, tag="ps", name="jp2")
        for i in range(10):
            nc.tensor.matmul(jp2[:], junk_w[:], junk_x[:], start=True, stop=True)
        emit_stage23(1)

    nc.compile()
    return nc


def _get_nc():
    key = (MM_DTYPE, S3_DTYPE)
    if key not in _cache:
        _cache[key] = _build(*key)
    return _cache[key]


def kernel(result_ls, result_A, result_lm, result_AT, result_ds, result_dm, W, b, h_n):
    global LAST_EXEC_TIME_NS, LAST_RESULT
    _install_ntff_hook()
    from concourse.bass_utils import run_bass_kernel_spmd

    import concourse.mybir as mybir

    nc = _get_nc()
    mm_np = mybir.dt.np(getattr(mybir.dt, MM_DTYPE))
    s3_np = mybir.dt.np(getattr(mybir.dt, S3_DTYPE))

    x = np.concatenate(
        [
            np.asarray(t, dtype=np.float32).reshape(B_TOTAL, -1)
            for t in (result_ls, result_A, result_lm, result_AT, result_ds, result_dm)
        ],
        axis=1,
    )  # [8192, 4096]
    W = np.asarray(W, dtype=np.float32)
    b = np.asarray(b, dtype=np.float32)
    h_n = np.asarray(h_n, dtype=np.float32)

    wT = np.ascontiguousarray(W[:, :DW].T).astype(mm_np)       # [1024, 512]
    wt = wT.reshape(2, 4, 128, OUT).transpose(0, 2, 1, 3)      # [2, 128, 4, 512]
    wt = np.ascontiguousarray(wt)
    bias = np.ascontiguousarray(b.reshape(4, 128).T)           # [128, 4]
    # h6[p, n*4 + oc, c] = h[oc*128 + p] if c == n else 0
    h6 = np.zeros((128, 24, 128), dtype=np.float32)
    for n in range(NODES):
        for oc in range(4):
            h6[:, n * 4 + oc, n] = h_n[oc * 128 : (oc + 1) * 128, 0]
    h6 = h6.astype(mm_np)
    eye = np.eye(6, dtype=np.float32)
    eye128 = np.eye(128, dtype=s3_np)

    in_maps = []
    for c in range(NCORES):
        xc = x[c * BLOC : (c + 1) * BLOC]                      # [1024, 4096]
        xT = xc.T                                              # [4096, 1024]
        xt = np.empty((NBC, NG, 128, GK, BC), dtype=mm_np)
        for bc in range(NBC):
            blk = xT[:, bc * BC : (bc + 1) * BC].reshape(NG, GK, 128, BC)
            xt[bc] = blk.transpose(0, 2, 1, 3)
        in_maps.append(
            {
                "xt": xt,
                "xb": np.ascontiguousarray(xc.reshape(8, 128, 4096)).astype(s3_np),
                "wt": wt.reshape(2, 128, 4 * OUT),
                "bias": bias,
                "h6": h6,
                "eye": eye,
                "eye128": eye128,
            }
        )

    res = run_bass_kernel_spmd(nc, in_maps, list(range(NCORES)))
    LAST_RESULT = res
    LAST_EXEC_TIME_NS = res.exec_time_ns

    out = np.zeros((B_TOTAL, 1, 2048), dtype=np.float32)
    for c in range(NCORES):
        zc = res.results[c]["z"]                               # [8, 128, 1024]
        out[c * BLOC : (c + 1) * BLOC, 0, :DW] = zc.reshape(BLOC, DW).astype(np.float32)
    return out


# revision 16
# speedup vs baseline: 1.2366x; 1.0452x over previous
"""Trainium2 Bass kernel for nn_Beta_score2 (gnn_message_passing).

Computation (per batch element b):
  nodes   = 6 feature vectors x_k (padded to 2048; padding never contributes)
  temp_k  = tanh(x_k @ W[:, :d_k]^T + b)          # [512]
  score_k = temp_k . h_n                           # scalar
  beta    = softmax(score)                         # [6]
  z       = sum_k beta_k * pad(x_k)                # [2048], cols 1024: always 0

Sharding: data-parallel over batch, B=8192 -> 1024 per core on 8 cores.

Per-core pipeline (two 512-wide batch chunks):
  stage 1: PE matmuls W^T-chunks x xT-chunks -> PSUM [128o, 512b];
           ACT fused bias+tanh -> temp^T in SBUF (bf16; PE runs bf16 at the
           full 2.4 GHz while fp16 is throttled ~20%).
  stage 2: score matmuls use 128-col zero-padded h-tiles (enables FWL fast
           weight load -> big-matmul speed) accumulating all 24 (node, oc)
           pieces into one PSUM [128, 512] whose rows 0:6 are the scores;
           PE-transpose to batch-major, softmax on ACT/DVE -> beta [128b, 24].
  stage 3: batch-major weighted sum with beta as per-partition scalars:
           ACT z = copy(x0 * b0), DVE scalar_tensor_tensor z += bk * xk.
           x for this stage (xb, fp16) is the natural row-major layout so its
           DMA is fully contiguous; z leaves batch-major (no host transpose).
  head: PE warm-up junk matmuls run during the initial DMA wait so the HAM
        clock gate releases before real matmuls; chunk-0 stage-2/3 emission
        is deferred past chunk-1's first node so the PE queue never blocks
        on the softmax transposes.

Host pre-tiles every DRAM tensor so each dma_start is a contiguous copy.
"""

import os
import sys
import types

import numpy as np

B_TOTAL = 8192
NCORES = 8
BLOC = B_TOTAL // NCORES  # 1024
OUT = 512
DW = 1024                 # only W[:, :1024] is ever used
NODES = 6
NODE_OFF = [0, 1024, 1536, 2048, 2560, 3584]
NODE_DIM = [1024, 512, 512, 512, 1024, 512]
NODE_ORDER = (1, 2, 0, 4, 3, 5)   # 1 first (single x-group), then 8-kc nodes
                                  # so the tanh pipeline gets slack
BC = 512                  # batch chunk on the free dim
NBC = BLOC // BC
GK = 4                    # xT group: [128, GK, BC]
NG = 8                    # 32 kc-chunks / GK

MM_DTYPE = os.environ.get("KERNEL_MM_DTYPE", "bfloat16")
S3_DTYPE = os.environ.get("KERNEL_S3_DTYPE", "float16")
N_JUNK = int(os.environ.get("KERNEL_N_JUNK", "11"))

LAST_EXEC_TIME_NS = None
LAST_RESULT = None

_cache = {}


def _install_ntff_hook():
    """run_bass_kernel_spmd(trace=True) under axon needs antenv.axon_hooks,
    which this image lacks; synthesize it from trn_agent_boot."""
    if "antenv.axon_hooks" in sys.modules:
        return
    try:
        import antenv
        import trn_agent_boot.trn_boot as tb
    except Exception:
        return
    mod = types.ModuleType("antenv.axon_hooks")
    _hook = tb._ntff_profile_via_ctypes("/opt/axon/libaxon_pjrt.so")
    mod.get_axon_ntff_profile_hook = lambda: _hook
    mod.set_axon_ntff_profile_hook = lambda h: None
    sys.modules["antenv.axon_hooks"] = mod
    antenv.axon_hooks = mod


def _build(mm_dtype_name, s3_dtype_name):
    from contextlib import ExitStack

    import concourse.bacc as bacc
    import concourse.mybir as mybir
    import concourse.tile as tile

    f32 = mybir.dt.float32
    mm_dt = getattr(mybir.dt, mm_dtype_name)
    s3_dt = getattr(mybir.dt, s3_dtype_name)

    nc = bacc.Bacc("TRN2", target_bir_lowering=False, debug=False)
    # pre-tiled inputs: every dma_start below is a contiguous copy
    xt_d = nc.dram_tensor("xt", [NBC, NG, 128, GK, BC], mm_dt, kind="ExternalInput").ap()
    xb_d = nc.dram_tensor("xb", [8, 128, 4096], s3_dt, kind="ExternalInput").ap()
    wt_d = nc.dram_tensor("wt", [2, 128, 4 * OUT], mm_dt, kind="ExternalInput").ap()
    bias_d = nc.dram_tensor("bias", [128, 4], f32, kind="ExternalInput").ap()
    h6_d = nc.dram_tensor("h6", [128, 24, 128], mm_dt, kind="ExternalInput").ap()
    eye_d = nc.dram_tensor("eye", [6, 6], f32, kind="ExternalInput").ap()
    eye128_d = nc.dram_tensor("eye128", [128, 128], s3_dt, kind="ExternalInput").ap()
    z_d = nc.dram_tensor("z", [8, 128, DW], s3_dt, kind="ExternalOutput").ap()

    Tanh = mybir.ActivationFunctionType.Tanh
    Exp = mybir.ActivationFunctionType.Exp
    Copy = mybir.ActivationFunctionType.Copy
    Mult = mybir.AluOpType.mult
    Add = mybir.AluOpType.add

    with tile.TileContext(nc) as tc, ExitStack() as ctx:
        const = ctx.enter_context(tc.tile_pool(name="const", bufs=1))
        wt_lo = const.tile([128, 4, OUT], mm_dt)
        wt_hi = const.tile([128, 4, OUT], mm_dt)
        bias_t = const.tile([128, 4], f32)
        h6_t = const.tile([128, 24, 128], mm_dt)
        eye_t = const.tile([6, 6], f32)
        eye128_t = const.tile([128, 128], s3_dt)
        junk_w = const.tile([128, 128], mm_dt, name="junk_w")
        junk_x = const.tile([128, BC], mm_dt, name="junk_x")

        pre_ps = ctx.enter_context(tc.tile_pool(name="pre", bufs=4, space="PSUM"))
        score_ps = ctx.enter_context(tc.tile_pool(name="score", bufs=2, space="PSUM"))
        tp_ps = ctx.enter_context(tc.tile_pool(name="tp", bufs=1, space="PSUM"))
        temps = ctx.enter_context(tc.tile_pool(name="temps", bufs=4))
        small = ctx.enter_context(tc.tile_pool(name="small", bufs=2))
        zpool = ctx.enter_context(tc.tile_pool(name="zpool", bufs=3))
        diag_pool = ctx.enter_context(tc.tile_pool(name="diag", bufs=12))

        # ---- PE warm-up: junk matmuls release the HAM clock gate while the
        # first DMAs are in flight.  memsets on GpSimd (idle at start).
        nc.gpsimd.memset(junk_w[:], 0.0)
        nc.gpsimd.memset(junk_x[:], 0.0)
        jp_a = pre_ps.tile([128, BC], f32, name="jp_a", tag="ps")
        jp = score_ps.tile([128, BC], f32, name="junk_ps", tag="sc")
        for i in range(4):
            nc.tensor.matmul(jp_a[:], junk_w[:], junk_x[:], start=True, stop=True)
        for i in range(N_JUNK - 4):
            nc.tensor.matmul(jp[:], junk_w[:], junk_x[:], start=True, stop=True)
        # dummy tanh pulls the 1.3us ACT table load into the DMA-wait window
        warm_t = temps.tile([128, BC], mm_dt, tag="tt", name="warm_t")
        nc.scalar.activation(warm_t[:], junk_x[:], Tanh, bias=0.0, scale=1.0)

        # ---- DMA dispatch. Critical path on Sync; bulk x streams on GpSimd.
        xts_tiles = {}

        def load_xt(bc, g, eng):
            t = const.tile([128, GK, BC], mm_dt, name=f"xt_{bc}_{g}")
            eng.dma_start(t[:], xt_d[bc, g])
            xts_tiles[(g, bc)] = t

        xb_tiles = {}

        def load_xb(j, eng):
            t = const.tile([128, 4096], s3_dt, name=f"xb_{j}")
            eng.dma_start(t[:], xb_d[j])
            xb_tiles[j] = t

        # Critical tiles ride alone on the Sync queue (MM#1's semaphore wait
        # is batched per-queue) and get the DMA bus to themselves until the
        # 4th junk matmul lifts the gate; everything else streams on GpSimd
        # in exact consumption order.
        nc.sync.dma_start(wt_lo[:], wt_d[0])
        load_xt(0, 2, nc.sync)
        nc.sync.dma_start(bias_t[:], bias_d[:, :])
        gate1 = const.tile([1, 1], f32, name="gate1")
        gate2 = const.tile([1, 1], f32, name="gate2")
        nc.vector.tensor_copy(gate1[:], jp_a[0:1, 0:1])
        nc.gpsimd.tensor_copy(gate2[:], gate1[:])
        load_xt(0, 3, nc.gpsimd)
        nc.gpsimd.dma_start(wt_hi[:], wt_d[1])
        nc.gpsimd.dma_start(h6_t[:], h6_d[:, :, :])
        for g in (0, 1, 5, 6, 4, 7):
            load_xt(0, g, nc.gpsimd)
        nc.gpsimd.dma_start(eye_t[:], eye_d[:, :])
        nc.gpsimd.dma_start(eye128_t[:], eye128_d[:, :])
        for g in (2, 3, 0, 1):
            load_xt(1, g, nc.gpsimd)
        for j in range(4):
            load_xb(j, nc.gpsimd)
        for g in (5, 6, 4, 7):
            load_xt(1, g, nc.gpsimd)
        for j in range(4, 8):
            load_xb(j, nc.gpsimd)

        def xts(kc, bc):
            return xts_tiles[(kc // GK, bc)][:, kc % GK, :]

        def wts(kc, oc):
            w = wt_lo if kc < 4 else wt_hi
            return w[:, kc % 4, oc * 128 : (oc + 1) * 128]

        # Score matmuls are emitted one group late ("pending"), so the PE
        # always has the next group's main matmuls queued between a tanh and
        # the score matmul that consumes it.
        pending_sc = []

        def flush_sc():
            if pending_sc:
                sc_t, lhsT, rhs, st, sp = pending_sc.pop()
                nc.tensor.matmul(sc_t[:], lhsT, rhs, start=st, stop=sp)

        scs = {}

        def emit_stage1_node(bc, n):
            if n == NODE_ORDER[0]:
                scs[bc] = score_ps.tile([128, BC], f32, name=f"sc_{bc}", tag="sc")
            sc = scs[bc]
            nk = NODE_DIM[n] // 128
            off = NODE_OFF[n] // 128
            for oc in range(4):
                ps = pre_ps.tile([128, BC], f32)
                for kc in range(nk):
                    nc.tensor.matmul(
                        ps[:],
                        wts(kc, oc),
                        xts(off + kc, bc),
                        start=(kc == 0),
                        stop=(kc == nk - 1),
                    )
                tt = temps.tile([128, BC], mm_dt, tag="tt", name="tt")
                nc.scalar.activation(
                    tt[:], ps[:], Tanh, bias=bias_t[:, oc : oc + 1], scale=1.0
                )
                flush_sc()
                pending_sc.append(
                    (
                        sc,
                        h6_t[:, n * 4 + oc, :],
                        tt[:],
                        (n == NODE_ORDER[0] and oc == 0),
                        (n == NODE_ORDER[-1] and oc == 3),
                    )
                )

        def emit_stage23(bc):
            sc = scs[bc]
            # ---------- softmax over the 6 nodes (batch-major) ----------
            sc_sb = small.tile([6, BC], f32, tag="sc_sb")
            nc.scalar.copy(sc_sb[:], sc[0:6, :])
            tp = tp_ps.tile([128, 24], f32)
            for j in range(4):
                nc.tensor.transpose(
                    tp[:, j * 6 : (j + 1) * 6],
                    sc_sb[:, j * 128 : (j + 1) * 128],
                    eye_t[:],
                )
            expt = small.tile([128, 24], f32, tag="expt")
            sumexp = small.tile([128, 4], f32, tag="sumexp")
            nc.scalar.activation(expt[:], tp[:], Exp)
            nc.vector.tensor_reduce(
                sumexp[:],
                expt[:].rearrange("p (j k) -> p j k", j=4),
                axis=mybir.AxisListType.X,
                op=mybir.AluOpType.add,
            )
            rec = small.tile([128, 4], f32, tag="rec")
            nc.vector.reciprocal(rec[:], sumexp[:])
            beta = small.tile([128, 24], f32, tag="beta")
            for j in range(4):
                nc.vector.tensor_scalar_mul(
                    beta[:, j * 6 : (j + 1) * 6],
                    expt[:, j * 6 : (j + 1) * 6],
                    rec[:, j : j + 1],
                )
            # ---------- stage 3: batch-major z = sum_k beta_k * x_k ----------
            def dve_group(j):
                # ACT z-init + DVE FMA chain
                xb = xb_tiles[bc * 4 + j]
                bf = beta[:, j * 6 : j * 6 + 6]
                z = zpool.tile([128, DW], s3_dt, tag="z", name=f"z_{bc}_{j}")
                nc.scalar.activation(z[:], xb[:, 0:DW], Copy, scale=bf[:, 0:1])
                nc.vector.scalar_tensor_tensor(
                    z[:], xb[:, 2560:3584], bf[:, 4:5], z[:], Mult, Add
                )
                for k, lo in ((1, 1024), (2, 1536), (3, 2048), (5, 3584)):
                    nc.vector.scalar_tensor_tensor(
                        z[:, 0:512], xb[:, lo : lo + 512], bf[:, k : k + 1],
                        z[:, 0:512], Mult, Add,
                    )
                nc.sync.dma_start(z_d[bc * 4 + j], z[:])

            if bc < NBC - 1:
                for j in range(4):
                    dve_group(j)
            else:
                # tail chunk: j3 on ACT+DVE; j0/j1/j2 as PE diag-matmuls
                # (diag tiles for j0 built on DVE, j1/j2 on ACT); copies
                # back from PSUM split ACT (j0, j2) / DVE (j1).
                dve_group(3)

                def build_diags(j, eng):
                    bf = beta[:, j * 6 : j * 6 + 6]
                    diags = []
                    for k in range(6):
                        d = diag_pool.tile([128, 128], s3_dt, tag="dg", name=f"dg{j}_{k}")
                        if eng == "dve":
                            nc.vector.tensor_scalar_mul(
                                d[:], eye128_t[:], bf[:, k : k + 1]
                            )
                        else:
                            nc.scalar.activation(
                                d[:], eye128_t[:], Copy, scale=bf[:, k : k + 1]
                            )
                        diags.append(d)
                    return diags

                def pe_group_mm(j, diags):
                    xb = xb_tiles[bc * 4 + j]
                    za = pre_ps.tile([128, BC], f32, tag="ps", name=f"za_{j}")
                    for i, (k, lo) in enumerate(
                        ((0, 0), (1, 1024), (2, 1536), (3, 2048), (4, 2560), (5, 3584))
                    ):
                        nc.tensor.matmul(
                            za[:], diags[k][:], xb[:, lo : lo + 512],
                            start=(i == 0), stop=(i == 5),
                        )
                    zb = pre_ps.tile([128, BC], f32, tag="ps", name=f"zb_{j}")
                    nc.tensor.matmul(
                        zb[:], diags[0][:], xb[:, 512:1024], start=True, stop=False
                    )
                    nc.tensor.matmul(
                        zb[:], diags[4][:], xb[:, 3072:3584], start=False, stop=True
                    )
                    return za, zb

                def pe_group_out(j, za, zb, eng):
                    z = zpool.tile([128, DW], s3_dt, tag="z", name=f"zd_{j}")
                    if eng == "dve":
                        nc.vector.tensor_copy(z[:, 0:512], za[:])
                        nc.vector.tensor_copy(z[:, 512:1024], zb[:])
                    else:
                        nc.scalar.copy(z[:, 0:512], za[:])
                        nc.scalar.copy(z[:, 512:1024], zb[:])
                    nc.sync.dma_start(z_d[bc * 4 + j], z[:])

                d0 = build_diags(0, "dve")
                za0, zb0 = pe_group_mm(0, d0)
                d1 = build_diags(1, "dve")
                za1, zb1 = pe_group_mm(1, d1)
                d2 = build_diags(2, "act")
                za2, zb2 = pe_group_mm(2, d2)
                pe_group_out(0, za0, zb0, "act")
                pe_group_out(1, za1, zb1, "act")
                pe_group_out(2, za2, zb2, "act")

        # chunk 0 stage 1; defer its stage 2/3 past chunk 1's first node so
        # the softmax transposes never block the PE queue.
        for n in NODE_ORDER:
            emit_stage1_node(0, n)
        emit_stage1_node(1, NODE_ORDER[0])
        emit_stage23(0)
        for n in NODE_ORDER[1:]:
            emit_stage1_node(1, n)
        flush_sc()
        # keep the PE busy through the softmax window so the HAM clock gate
        # stays open for the tail diag-matmuls
        jp2 = pre_ps.tile([128, BC], f32, tag="ps", name="jp2")
        for i in range(10):
            nc.tensor.matmul(jp2[:], junk_w[:], junk_x[:], start=True, stop=True)
        emit_stage23(1)

    nc.compile()
    return nc


def _get_nc():
    key = (MM_DTYPE, S3_DTYPE)
    if key not in _cache:
        _cache[key] = _build(*key)
    return _cache[key]


def kernel(result_ls, result_A, result_lm, result_AT, result_ds, result_dm, W, b, h_n):
    global LAST_EXEC_TIME_NS, LAST_RESULT
    _install_ntff_hook()
    from concourse.bass_utils import run_bass_kernel_spmd

    import concourse.mybir as mybir

    nc = _get_nc()
    mm_np = mybir.dt.np(getattr(mybir.dt, MM_DTYPE))
    s3_np = mybir.dt.np(getattr(mybir.dt, S3_DTYPE))

    x = np.concatenate(
        [
            np.asarray(t, dtype=np.float32).reshape(B_TOTAL, -1)
            for t in (result_ls, result_A, result_lm, result_AT, result_ds, result_dm)
        ],
        axis=1,
    )  # [8192, 4096]
    W = np.asarray(W, dtype=np.float32)
    b = np.asarray(b, dtype=np.float32)
    h_n = np.asarray(h_n, dtype=np.float32)

    wT = np.ascontiguousarray(W[:, :DW].T).astype(mm_np)       # [1024, 512]
    wt = wT.reshape(2, 4, 128, OUT).transpose(0, 2, 1, 3)      # [2, 128, 4, 512]
    wt = np.ascontiguousarray(wt)
    bias = np.ascontiguousarray(b.reshape(4, 128).T)           # [128, 4]
    # h6[p, n*4 + oc, c] = h[oc*128 + p] if c == n else 0
    h6 = np.zeros((128, 24, 128), dtype=np.float32)
    for n in range(NODES):
        for oc in range(4):
            h6[:, n * 4 + oc, n] = h_n[oc * 128 : (oc + 1) * 128, 0]
    h6 = h6.astype(mm_np)
    eye = np.eye(6, dtype=np.float32)
    eye128 = np.eye(128, dtype=s3_np)

    in_maps = []
    for c in range(NCORES):
        xc = x[c * BLOC : (c + 1) * BLOC]                      # [1024, 4096]
        xT = xc.T                                              # [4096, 1024]
        xt = np.empty((NBC, NG, 128, GK, BC), dtype=mm_np)
        for bc in range(NBC):
            blk = xT[:, bc * BC : (bc + 1) * BC].reshape(NG, GK, 128, BC)
            xt[bc] = blk.transpose(0, 2, 1, 3)
        in_maps.append(
            {
                "xt": xt,
                "xb": np.ascontiguousarray(xc.reshape(8, 128, 4096)).astype(s3_np),
                "wt": wt.reshape(2, 128, 4 * OUT),
                "bias": bias,
                "h6": h6,
                "eye": eye,
                "eye128": eye128,
            }
        )

    res = run_bass_kernel_spmd(nc, in_maps, list(range(NCORES)))
    LAST_RESULT = res
    LAST_EXEC_TIME_NS = res.exec_time_ns

    out = np.zeros((B_TOTAL, 1, 2048), dtype=np.float32)
    for c in range(NCORES):
        zc = res.results[c]["z"]                               # [8, 128, 1024]
        out[c * BLOC : (c + 1) * BLOC, 0, :DW] = zc.reshape(BLOC, DW).astype(np.float32)
    return out


# revision 17
# speedup vs baseline: 1.2495x; 1.0105x over previous
"""Trainium2 Bass kernel for nn_Beta_score2 (gnn_message_passing).

Computation (per batch element b):
  nodes   = 6 feature vectors x_k (padded to 2048; padding never contributes)
  temp_k  = tanh(x_k @ W[:, :d_k]^T + b)          # [512]
  score_k = temp_k . h_n                           # scalar
  beta    = softmax(score)                         # [6]
  z       = sum_k beta_k * pad(x_k)                # [2048], cols 1024: always 0

Sharding: data-parallel over batch, B=8192 -> 1024 per core on 8 cores.

Per-core pipeline (two 512-wide batch chunks):
  stage 1: PE matmuls W^T-chunks x xT-chunks -> PSUM [128o, 512b];
           ACT fused bias+tanh -> temp^T in SBUF (bf16; PE runs bf16 at the
           full 2.4 GHz while fp16 is throttled ~20%).
  stage 2: score matmuls use 128-col zero-padded h-tiles (enables FWL fast
           weight load -> big-matmul speed) accumulating all 24 (node, oc)
           pieces into one PSUM [128, 512] whose rows 0:6 are the scores;
           PE-transpose to batch-major, softmax on ACT/DVE -> beta [128b, 24].
  stage 3: batch-major weighted sum with beta as per-partition scalars:
           ACT z = copy(x0 * b0), DVE scalar_tensor_tensor z += bk * xk.
           x for this stage (xb, fp16) is the natural row-major layout so its
           DMA is fully contiguous; z leaves batch-major (no host transpose).
  head: PE warm-up junk matmuls run during the initial DMA wait so the HAM
        clock gate releases before real matmuls; chunk-0 stage-2/3 emission
        is deferred past chunk-1's first node so the PE queue never blocks
        on the softmax transposes.

Host pre-tiles every DRAM tensor so each dma_start is a contiguous copy.
"""

import os
import sys
import types

import numpy as np

B_TOTAL = 8192
NCORES = 8
BLOC = B_TOTAL // NCORES  # 1024
OUT = 512
DW = 1024                 # only W[:, :1024] is ever used
NODES = 6
NODE_OFF = [0, 1024, 1536, 2048, 2560, 3584]
NODE_DIM = [1024, 512, 512, 512, 1024, 512]
NODE_ORDER = (1, 2, 0, 4, 3, 5)   # 1 first (single x-group), then 8-kc nodes
                                  # so the tanh pipeline gets slack
BC = 512                  # batch chunk on the free dim
NBC = BLOC // BC
GK = 4                    # xT group: [128, GK, BC]
NG = 8                    # 32 kc-chunks / GK

MM_DTYPE = os.environ.get("KERNEL_MM_DTYPE", "bfloat16")
S3_DTYPE = os.environ.get("KERNEL_S3_DTYPE", "float16")
N_JUNK = int(os.environ.get("KERNEL_N_JUNK", "14"))

LAST_EXEC_TIME_NS = None
LAST_RESULT = None

_cache = {}


def _install_ntff_hook():
    """run_bass_kernel_spmd(trace=True) under axon needs antenv.axon_hooks,
    which this image lacks; synthesize it from trn_agent_boot."""
    if "antenv.axon_hooks" in sys.modules:
        return
    try:
        import antenv
        import trn_agent_boot.trn_boot as tb
    except Exception:
        return
    mod = types.ModuleType("antenv.axon_hooks")
    _hook = tb._ntff_profile_via_ctypes("/opt/axon/libaxon_pjrt.so")
    mod.get_axon_ntff_profile_hook = lambda: _hook
    mod.set_axon_ntff_profile_hook = lambda h: None
    sys.modules["antenv.axon_hooks"] = mod
    antenv.axon_hooks = mod


def _build(mm_dtype_name, s3_dtype_name):
    from contextlib import ExitStack

    import concourse.bacc as bacc
    import concourse.mybir as mybir
    import concourse.tile as tile

    f32 = mybir.dt.float32
    mm_dt = getattr(mybir.dt, mm_dtype_name)
    s3_dt = getattr(mybir.dt, s3_dtype_name)

    nc = bacc.Bacc("TRN2", target_bir_lowering=False, debug=False)
    # pre-tiled inputs: every dma_start below is a contiguous copy
    xt_d = nc.dram_tensor("xt", [NBC, NG, 128, GK, BC], mm_dt, kind="ExternalInput").ap()
    xb_d = nc.dram_tensor("xb", [8, 128, 4096], s3_dt, kind="ExternalInput").ap()
    wt_d = nc.dram_tensor("wt", [2, 128, 4 * OUT], mm_dt, kind="ExternalInput").ap()
    bias_d = nc.dram_tensor("bias", [128, 4], f32, kind="ExternalInput").ap()
    h6_d = nc.dram_tensor("h6", [128, 24, 128], mm_dt, kind="ExternalInput").ap()
    eye_d = nc.dram_tensor("eye", [6, 6], f32, kind="ExternalInput").ap()
    eye128_d = nc.dram_tensor("eye128", [128, 128], s3_dt, kind="ExternalInput").ap()
    z_d = nc.dram_tensor("z", [8, 128, DW], s3_dt, kind="ExternalOutput").ap()

    Tanh = mybir.ActivationFunctionType.Tanh
    Exp = mybir.ActivationFunctionType.Exp
    Copy = mybir.ActivationFunctionType.Copy
    Mult = mybir.AluOpType.mult
    Add = mybir.AluOpType.add

    with tile.TileContext(nc) as tc, ExitStack() as ctx:
        const = ctx.enter_context(tc.tile_pool(name="const", bufs=1))
        wt_lo = const.tile([128, 4, OUT], mm_dt)
        wt_hi = const.tile([128, 4, OUT], mm_dt)
        bias_t = const.tile([128, 4], f32)
        h6_t = const.tile([128, 24, 128], mm_dt)
        eye_t = const.tile([6, 6], f32)
        eye128_t = const.tile([128, 128], s3_dt)
        junk_w = const.tile([128, 128], mm_dt, name="junk_w")
        junk_x = const.tile([128, BC], mm_dt, name="junk_x")

        pre_ps = ctx.enter_context(tc.tile_pool(name="pre", bufs=4, space="PSUM"))
        score_ps = ctx.enter_context(tc.tile_pool(name="score", bufs=2, space="PSUM"))
        tp_ps = ctx.enter_context(tc.tile_pool(name="tp", bufs=1, space="PSUM"))
        temps = ctx.enter_context(tc.tile_pool(name="temps", bufs=4))
        small = ctx.enter_context(tc.tile_pool(name="small", bufs=2))
        zpool = ctx.enter_context(tc.tile_pool(name="zpool", bufs=3))
        diag_pool = ctx.enter_context(tc.tile_pool(name="diag", bufs=12))

        # ---- PE warm-up: junk matmuls release the HAM clock gate while the
        # first DMAs are in flight.  memsets on GpSimd (idle at start).
        nc.gpsimd.memset(junk_w[:], 0.0)
        nc.gpsimd.memset(junk_x[:], 0.0)
        jp_a = pre_ps.tile([128, BC], f32, name="jp_a", tag="ps")
        jp = score_ps.tile([128, BC], f32, name="junk_ps", tag="sc")
        for i in range(4):
            nc.tensor.matmul(jp_a[:], junk_w[:], junk_x[:], start=True, stop=True)
        for i in range(N_JUNK - 4):
            nc.tensor.matmul(jp[:], junk_w[:], junk_x[:], start=True, stop=True)
        # dummy tanh pulls the 1.3us ACT table load into the DMA-wait window
        warm_t = temps.tile([128, BC], mm_dt, tag="tt", name="warm_t")
        nc.scalar.activation(warm_t[:], junk_x[:], Tanh, bias=0.0, scale=1.0)

        # ---- DMA dispatch. Critical path on Sync; bulk x streams on GpSimd.
        xts_tiles = {}

        def load_xt(bc, g, eng):
            t = const.tile([128, GK, BC], mm_dt, name=f"xt_{bc}_{g}")
            eng.dma_start(t[:], xt_d[bc, g])
            xts_tiles[(g, bc)] = t

        xb_tiles = {}

        def load_xb(j, eng):
            t = const.tile([128, 4096], s3_dt, name=f"xb_{j}")
            eng.dma_start(t[:], xb_d[j])
            xb_tiles[j] = t

        # Critical tiles ride alone on the Sync queue (MM#1's semaphore wait
        # is batched per-queue) and get the DMA bus to themselves until the
        # 4th junk matmul lifts the gate; everything else streams on GpSimd
        # in exact consumption order.
        nc.sync.dma_start(wt_lo[:], wt_d[0])
        load_xt(0, 2, nc.sync)
        nc.sync.dma_start(bias_t[:], bias_d[:, :])
        gate1 = const.tile([1, 1], f32, name="gate1")
        gate2 = const.tile([1, 1], f32, name="gate2")
        nc.vector.tensor_copy(gate1[:], jp_a[0:1, 0:1])
        nc.gpsimd.tensor_copy(gate2[:], gate1[:])
        load_xt(0, 3, nc.gpsimd)
        nc.gpsimd.dma_start(wt_hi[:], wt_d[1])
        nc.gpsimd.dma_start(h6_t[:], h6_d[:, :, :])
        for g in (0, 1, 5, 6, 4, 7):
            load_xt(0, g, nc.gpsimd)
        nc.gpsimd.dma_start(eye_t[:], eye_d[:, :])
        nc.gpsimd.dma_start(eye128_t[:], eye128_d[:, :])
        for g in (2, 3, 0, 1):
            load_xt(1, g, nc.gpsimd)
        for j in range(4):
            load_xb(j, nc.gpsimd)
        for g in (5, 6, 4, 7):
            load_xt(1, g, nc.gpsimd)
        for j in range(4, 8):
            load_xb(j, nc.gpsimd)

        def xts(kc, bc):
            return xts_tiles[(kc // GK, bc)][:, kc % GK, :]

        def wts(kc, oc):
            w = wt_lo if kc < 4 else wt_hi
            return w[:, kc % 4, oc * 128 : (oc + 1) * 128]

        # Score matmuls are emitted one group late ("pending"), so the PE
        # always has the next group's main matmuls queued between a tanh and
        # the score matmul that consumes it.
        pending_sc = []

        def flush_sc():
            if pending_sc:
                sc_t, lhsT, rhs, st, sp = pending_sc.pop()
                nc.tensor.matmul(sc_t[:], lhsT, rhs, start=st, stop=sp)

        scs = {}

        def emit_stage1_node(bc, n):
            if n == NODE_ORDER[0]:
                scs[bc] = score_ps.tile([128, BC], f32, name=f"sc_{bc}", tag="sc")
            sc = scs[bc]
            nk = NODE_DIM[n] // 128
            off = NODE_OFF[n] // 128
            for oc in range(4):
                ps = pre_ps.tile([128, BC], f32)
                for kc in range(nk):
                    nc.tensor.matmul(
                        ps[:],
                        wts(kc, oc),
                        xts(off + kc, bc),
                        start=(kc == 0),
                        stop=(kc == nk - 1),
                    )
                tt = temps.tile([128, BC], mm_dt, tag="tt", name="tt")
                nc.scalar.activation(
                    tt[:], ps[:], Tanh, bias=bias_t[:, oc : oc + 1], scale=1.0
                )
                flush_sc()
                pending_sc.append(
                    (
                        sc,
                        h6_t[:, n * 4 + oc, :],
                        tt[:],
                        (n == NODE_ORDER[0] and oc == 0),
                        (n == NODE_ORDER[-1] and oc == 3),
                    )
                )

        def emit_stage23(bc):
            sc = scs[bc]
            # ---------- softmax over the 6 nodes (batch-major) ----------
            sc_sb = small.tile([6, BC], f32, tag="sc_sb")
            nc.scalar.copy(sc_sb[:], sc[0:6, :])
            tp = tp_ps.tile([128, 24], f32)
            for j in range(4):
                nc.tensor.transpose(
                    tp[:, j * 6 : (j + 1) * 6],
                    sc_sb[:, j * 128 : (j + 1) * 128],
                    eye_t[:],
                )
            expt = small.tile([128, 24], f32, tag="expt")
            sumexp = small.tile([128, 4], f32, tag="sumexp")
            nc.scalar.activation(expt[:], tp[:], Exp)
            nc.vector.tensor_reduce(
                sumexp[:],
                expt[:].rearrange("p (j k) -> p j k", j=4),
                axis=mybir.AxisListType.X,
                op=mybir.AluOpType.add,
            )
            rec = small.tile([128, 4], f32, tag="rec")
            nc.vector.reciprocal(rec[:], sumexp[:])
            beta = small.tile([128, 24], f32, tag="beta")
            for j in range(4):
                nc.vector.tensor_scalar_mul(
                    beta[:, j * 6 : (j + 1) * 6],
                    expt[:, j * 6 : (j + 1) * 6],
                    rec[:, j : j + 1],
                )
            # ---------- stage 3: batch-major z = sum_k beta_k * x_k ----------
            def dve_group(j):
                # ACT z-init + DVE FMA chain
                xb = xb_tiles[bc * 4 + j]
                bf = beta[:, j * 6 : j * 6 + 6]
                z = zpool.tile([128, DW], s3_dt, tag="z", name=f"z_{bc}_{j}")
                nc.scalar.activation(z[:], xb[:, 0:DW], Copy, scale=bf[:, 0:1])
                nc.vector.scalar_tensor_tensor(
                    z[:], xb[:, 2560:3584], bf[:, 4:5], z[:], Mult, Add
                )
                for k, lo in ((1, 1024), (2, 1536), (3, 2048), (5, 3584)):
                    nc.vector.scalar_tensor_tensor(
                        z[:, 0:512], xb[:, lo : lo + 512], bf[:, k : k + 1],
                        z[:, 0:512], Mult, Add,
                    )
                nc.sync.dma_start(z_d[bc * 4 + j], z[:])

            if bc < NBC - 1:
                for j in range(4):
                    dve_group(j)
            else:
                # tail chunk: j3 on ACT+DVE; j0/j1/j2 as PE diag-matmuls
                # (diag tiles for j0 built on DVE, j1/j2 on ACT); copies
                # back from PSUM split ACT (j0, j2) / DVE (j1).
                dve_group(3)

                def build_diags(j, eng):
                    bf = beta[:, j * 6 : j * 6 + 6]
                    diags = []
                    for k in range(6):
                        d = diag_pool.tile([128, 128], s3_dt, tag="dg", name=f"dg{j}_{k}")
                        if eng == "dve":
                            nc.vector.tensor_scalar_mul(
                                d[:], eye128_t[:], bf[:, k : k + 1]
                            )
                        else:
                            nc.scalar.activation(
                                d[:], eye128_t[:], Copy, scale=bf[:, k : k + 1]
                            )
                        diags.append(d)
                    return diags

                def pe_group_mm(j, diags):
                    xb = xb_tiles[bc * 4 + j]
                    za = pre_ps.tile([128, BC], f32, tag="ps", name=f"za_{j}")
                    for i, (k, lo) in enumerate(
                        ((0, 0), (1, 1024), (2, 1536), (3, 2048), (4, 2560), (5, 3584))
                    ):
                        nc.tensor.matmul(
                            za[:], diags[k][:], xb[:, lo : lo + 512],
                            start=(i == 0), stop=(i == 5),
                        )
                    zb = pre_ps.tile([128, BC], f32, tag="ps", name=f"zb_{j}")
                    nc.tensor.matmul(
                        zb[:], diags[0][:], xb[:, 512:1024], start=True, stop=False
                    )
                    nc.tensor.matmul(
                        zb[:], diags[4][:], xb[:, 3072:3584], start=False, stop=True
                    )
                    return za, zb

                def pe_group_out(j, za, zb, eng):
                    z = zpool.tile([128, DW], s3_dt, tag="z", name=f"zd_{j}")
                    if eng == "dve":
                        nc.vector.tensor_copy(z[:, 0:512], za[:])
                        nc.vector.tensor_copy(z[:, 512:1024], zb[:])
                    else:
                        nc.scalar.copy(z[:, 0:512], za[:])
                        nc.scalar.copy(z[:, 512:1024], zb[:])
                    nc.sync.dma_start(z_d[bc * 4 + j], z[:])

                d0 = build_diags(0, "dve")
                za0, zb0 = pe_group_mm(0, d0)
                d1 = build_diags(1, "dve")
                za1, zb1 = pe_group_mm(1, d1)
                d2 = build_diags(2, "act")
                za2, zb2 = pe_group_mm(2, d2)
                pe_group_out(0, za0, zb0, "act")
                pe_group_out(1, za1, zb1, "act")
                pe_group_out(2, za2, zb2, "act")

        # chunk 0 stage 1; defer its stage 2/3 past chunk 1's first node so
        # the softmax transposes never block the PE queue.
        for n in NODE_ORDER:
            emit_stage1_node(0, n)
        emit_stage1_node(1, NODE_ORDER[0])
        emit_stage23(0)
        for n in NODE_ORDER[1:]:
            emit_stage1_node(1, n)
        flush_sc()
        # keep the PE busy through the softmax window so the HAM clock gate
        # stays open for the tail diag-matmuls
        jp2 = pre_ps.tile([128, BC], f32, tag="ps", name="jp2")
        for i in range(10):
            nc.tensor.matmul(jp2[:], junk_w[:], junk_x[:], start=True, stop=True)
        emit_stage23(1)

    nc.compile()
    return nc


def _get_nc():
    key = (MM_DTYPE, S3_DTYPE)
    if key not in _cache:
        _cache[key] = _build(*key)
    return _cache[key]


def kernel(result_ls, result_A, result_lm, result_AT, result_ds, result_dm, W, b, h_n):
    global LAST_EXEC_TIME_NS, LAST_RESULT
    _install_ntff_hook()
    from concourse.bass_utils import run_bass_kernel_spmd

    import concourse.mybir as mybir

    nc = _get_nc()
    mm_np = mybir.dt.np(getattr(mybir.dt, MM_DTYPE))
    s3_np = mybir.dt.np(getattr(mybir.dt, S3_DTYPE))

    x = np.concatenate(
        [
            np.asarray(t, dtype=np.float32).reshape(B_TOTAL, -1)
            for t in (result_ls, result_A, result_lm, result_AT, result_ds, result_dm)
        ],
        axis=1,
    )  # [8192, 4096]
    W = np.asarray(W, dtype=np.float32)
    b = np.asarray(b, dtype=np.float32)
    h_n = np.asarray(h_n, dtype=np.float32)

    wT = np.ascontiguousarray(W[:, :DW].T).astype(mm_np)       # [1024, 512]
    wt = wT.reshape(2, 4, 128, OUT).transpose(0, 2, 1, 3)      # [2, 128, 4, 512]
    wt = np.ascontiguousarray(wt)
    bias = np.ascontiguousarray(b.reshape(4, 128).T)           # [128, 4]
    # h6[p, n*4 + oc, c] = h[oc*128 + p] if c == n else 0
    h6 = np.zeros((128, 24, 128), dtype=np.float32)
    for n in range(NODES):
        for oc in range(4):
            h6[:, n * 4 + oc, n] = h_n[oc * 128 : (oc + 1) * 128, 0]
    h6 = h6.astype(mm_np)
    eye = np.eye(6, dtype=np.float32)
    eye128 = np.eye(128, dtype=s3_np)

    in_maps = []
    for c in range(NCORES):
        xc = x[c * BLOC : (c + 1) * BLOC]                      # [1024, 4096]
        xT = xc.T                                              # [4096, 1024]
        xt = np.empty((NBC, NG, 128, GK, BC), dtype=mm_np)
        for bc in range(NBC):
            blk = xT[:, bc * BC : (bc + 1) * BC].reshape(NG, GK, 128, BC)
            xt[bc] = blk.transpose(0, 2, 1, 3)
        in_maps.append(
            {
                "xt": xt,
                "xb": np.ascontiguousarray(xc.reshape(8, 128, 4096)).astype(s3_np),
                "wt": wt.reshape(2, 128, 4 * OUT),
                "bias": bias,
                "h6": h6,
                "eye": eye,
                "eye128": eye128,
            }
        )

    res = run_bass_kernel_spmd(nc, in_maps, list(range(NCORES)))
    LAST_RESULT = res
    LAST_EXEC_TIME_NS = res.exec_time_ns

    out = np.zeros((B_TOTAL, 1, 2048), dtype=np.float32)
    for c in range(NCORES):
        zc = res.results[c]["z"]                               # [8, 128, 1024]
        out[c * BLOC : (c + 1) * BLOC, 0, :DW] = zc.reshape(BLOC, DW).astype(np.float32)
    return out
